# revision 12
# baseline (speedup 1.0000x reference)
"""MoE (noisy top-k gating, Shazeer-style) Trainium2 kernel.

Strategy (expert parallelism, per the sharding hint):
  - Gating (x@w_gate, noisy logits, top-4, softmax) runs on host in fp32
    numpy: it is 0.02% of the FLOPs and produces the routing needed to
    shard ("all-to-all dispatch" done host-side since I/O is full anyway).
  - The 16 experts' weights are sharded 2-per-core across 8 NeuronCores.
    Each core receives, per expert, the dispatched tokens x^T [D, C]
    (zero-padded to the global max expert load C), computes
    hT = relu(W1^T-free-dim-major matmul) and y = (hT)^T @ W2 scaled by
    the per-token gate, in bf16 on the tensor engine with fp32 PSUM
    accumulation.
  - Host scatters per-expert outputs back (each token appears in exactly
    its top-4 experts' outputs) and adds the gates @ b2 term.

Shapes are hardcoded for B=4096, D=1024, H=4096, E=16, TOP_K=4.
"""

import numpy as np
import ml_dtypes

import concourse.bass as bass
import concourse.mybir as mybir
import concourse.tile as tile
from concourse import bacc
from concourse.bass import ds, ts
from concourse.bass_utils import run_bass_kernel_spmd
from concourse.kernels.tile_matmul import (
    ShapeInfo,
    composable_matmul_tile_kernel,
    dma_from_dram_kxm,
    dma_from_dram_kxn,
    dma_to_dram_mxn,
    k_pool_min_bufs_for_dim,
)

B, D, H, E, TOP_K, NCORES = 4096, 1024, 4096, 16, 4, 8
EPC = E // NCORES  # experts per core
BF16 = mybir.dt.bfloat16
F32 = mybir.dt.float32
AF = mybir.ActivationFunctionType

# Results of the last device run (exec_time_ns etc.), for test harnesses.
LAST_RESULTS = None


def _gating(x, noise, w_gate, w_noise, b_noise):
    """Mirror of the reference gating in fp32 numpy.

    Verified on the actual inputs: the top-4 sets match jax-CPU bitwise
    selection (min 4th/5th logit gap 5.7e-5 vs <2e-6 numeric diff).
    """
    clean = x @ w_gate
    stddev = np.logaddexp(0.0, x @ w_noise + b_noise).astype(np.float32)
    noisy = clean + noise * stddev
    order = np.argsort(-noisy, axis=1, kind="stable")[:, :TOP_K]
    top_vals = np.take_along_axis(noisy, order, axis=1)
    ex = np.exp(top_vals - top_vals.max(axis=1, keepdims=True))
    top_gates = (ex / ex.sum(axis=1, keepdims=True)).astype(np.float32)
    return order, top_gates


def _relu_bias_reducer(b1_sb):
    def f(nc, psum, sbuf, md):
        hb = md.m_tile_idx * md.m_subtiles + md.m_subtile_idx
        nc.scalar.activation(
            sbuf[:, 0, :], psum[:], AF.Relu, bias=b1_sb[:, hb : hb + 1]
        )

    return f


def _gate_scale_reducer(g_sb):
    def f(nc, psum, sbuf, md):
        mb = md.m_tile_idx * md.m_subtiles + md.m_subtile_idx
        nc.scalar.activation(
            sbuf[:, 0, :], psum[:], AF.Copy, scale=g_sb[:, mb : mb + 1]
        )

    return f


def _noop_consumer(nc, t, md):
    pass


def _c_tile(C):
    for t in (512, 384, 256, 128):
        if C % t == 0:
            return t
    raise AssertionError(C)


def _build_program(Cs):
    """Build the SPMD per-core program: 2 experts, each a 2-layer FFN.

    Per expert: xT and the intermediate hT stay SBUF-resident; W1/W2
    stream from HBM; layer-1 output is written straight into the hT SBUF
    buffer (mxn_subtile_producer), layer 2 reads it as kxm via a custom
    producer — no DRAM roundtrip for h, no re-DMA of xT.
    """
    from contextlib import ExitStack

    nc = bacc.Bacc(None, target_bir_lowering=False)
    in_names = {}
    out_names = {}
    with ExitStack() as ctx:
        tc = ctx.enter_context(tile.TileContext(nc))
        dram = ctx.enter_context(tc.tile_pool(name="dram", bufs=1, space="DRAM"))
        const = ctx.enter_context(tc.tile_pool(name="const", bufs=1))

        ins = {}
        outs = {}
        for j in range(EPC):
            C = Cs[j]
            ins[f"w1_{j}"] = dram.tile([D, H], BF16, kind="ExternalInput", name=f"w1_{j}")
            ins[f"w2_{j}"] = dram.tile([H, D], BF16, kind="ExternalInput", name=f"w2_{j}")
            ins[f"xt_{j}"] = dram.tile([D, C], BF16, kind="ExternalInput", name=f"xt_{j}")
            ins[f"b1_{j}"] = dram.tile([128, H // 128], F32, kind="ExternalInput", name=f"b1_{j}")
            ins[f"g_{j}"] = dram.tile([128, C // 128], F32, kind="ExternalInput", name=f"g_{j}")
            outs[f"y_{j}"] = dram.tile([C, D], F32, kind="ExternalOutput", name=f"y_{j}")

        for key, ap in ins.items():
            in_names[key] = ap.tensor.name
        for key, ap in outs.items():
            out_names[key] = ap.tensor.name

        # Standing buffers for both experts' xT (and their W1 stream pools)
        # so the second expert's inputs prefetch with no SBUF-address WAR
        # against the first expert's in-flight reads.
        xt_sbs = []
        w1_pools = []
        for j in range(EPC):
            C = Cs[j]
            xt_sbs.append(const.tile([128, D // 128, C], BF16, name=f"xts{j}"))
            w1_pools.append(
                ctx.enter_context(
                    tc.tile_pool(name=f"w1p{j}", bufs=k_pool_min_bufs_for_dim(D) + 1)
                )
            )

        def _load_xt(j):
            xt3d = ins[f"xt_{j}"].rearrange("(ko p) c -> p ko c", p=128)
            for kt in range(2):
                ks = ds(kt * (D // 256), D // 256)
                nc.sync.dma_start(xt_sbs[j][:, ks], xt3d[:, ks])

        _load_xt(0)

        for j in range(EPC):
            C = Cs[j]
            CT = _c_tile(C)  # exact N tile for layer 1
            b1_sb = const.tile([128, H // 128], F32, name=f"b1sb{j}")
            nc.sync.dma_start(b1_sb[:], ins[f"b1_{j}"][:])
            g_sb = const.tile([128, C // 128], F32, name=f"gsb{j}")
            nc.sync.dma_start(g_sb[:], ins[f"g_{j}"][:])

            with ExitStack() as ectx:
                pers = ectx.enter_context(
                    tc.tile_pool(name=f"pers{j}", bufs=1)
                )
                xt_sb = xt_sbs[j]
                hT_sb = pers.tile([128, H // 128, C], BF16, name=f"hts{j}")

                def xt_producer(nc_, md, xt_sb=xt_sb):
                    return xt_sb[
                        :,
                        ts(md.k_tile_idx, md.k_subtiles),
                        ds(md.n_tile_idx * md.n_tile, md.n_tile),
                    ]

                def hT_out_producer(nc_, md, hT_sb=hT_sb):
                    return hT_sb[
                        :,
                        ds(md.m_tile_idx * md.m_subtiles, md.m_subtiles),
                        ds(md.n_tile_idx * md.n_tile, md.n_tile),
                    ]

                def hT_kxm_producer(nc_, md, hT_sb=hT_sb):
                    return hT_sb[
                        :,
                        ts(md.k_tile_idx, md.k_subtiles),
                        ds(md.m_tile_idx * md.m_tile, md.m_tile),
                    ]

                # layer 1: hT[H, C] = relu(W1[D,H].T @ xT[D,C] + b1)
                tc.swap_default_side()
                with ExitStack() as mctx:
                    w1_producer, w1_shape = dma_from_dram_kxm(
                        w1_pools[j], ins[f"w1_{j}"][:]
                    )
                    composable_matmul_tile_kernel(
                        tc=tc,
                        kxm_shape=w1_shape,
                        kxn_shape=ShapeInfo(pdims=((128, D // 128),), fdims=(C,)),
                        output_type=BF16,
                        kxm_producer=w1_producer,
                        kxn_producer=xt_producer,
                        mxn_subtile_reducer=_relu_bias_reducer(b1_sb),
                        mxn_subtile_producer=hT_out_producer,
                        mxn_consumer=_noop_consumer,
                        MAX_TILE_SIZE=CT,
                        psum_n_bufs=2,
                    )

                # prefetch the next expert's xT while this expert computes
                if j + 1 < EPC:
                    _load_xt(j + 1)

                # layer 2: y[C, D] = g * (hT[H,C].T @ W2[H,D])
                tc.swap_default_side()
                with ExitStack() as mctx:
                    w2_pool = mctx.enter_context(
                        tc.tile_pool(
                            name=f"w2p{j}", bufs=k_pool_min_bufs_for_dim(H) + 1
                        )
                    )
                    w2_producer, w2_shape = dma_from_dram_kxn(
                        w2_pool, ins[f"w2_{j}"][:]
                    )
                    composable_matmul_tile_kernel(
                        tc=tc,
                        kxm_shape=ShapeInfo(pdims=((128, H // 128),), fdims=(C,)),
                        kxn_shape=w2_shape,
                        output_type=F32,
                        kxm_producer=hT_kxm_producer,
                        kxn_producer=w2_producer,
                        mxn_subtile_reducer=_gate_scale_reducer(g_sb),
                        mxn_consumer=dma_to_dram_mxn(outs[f"y_{j}"][:]),
                        psum_n_bufs=2,
                    )
    nc.compile()
    return nc, in_names, out_names


def kernel(x, noise, w_gate, w_noise, b_noise, W1, b1, W2, b2):
    global LAST_RESULTS
    x = np.asarray(x, np.float32)
    noise = np.asarray(noise, np.float32)
    w_gate = np.asarray(w_gate, np.float32)
    w_noise = np.asarray(w_noise, np.float32)
    b_noise = np.asarray(b_noise, np.float32)
    W1 = np.asarray(W1, np.float32)
    b1 = np.asarray(b1, np.float32)
    W2 = np.asarray(W2, np.float32)
    b2 = np.asarray(b2, np.float32)

    # ---- host gating + dispatch ----
    top_idx, top_gates = _gating(x, noise, w_gate, w_noise, b_noise)

    counts = np.bincount(top_idx.ravel(), minlength=E)

    # Slot assignment: rank experts by load; the 8 heaviest go to slot 0,
    # the 8 lightest to slot 1, so slot 1's padded capacity is smaller.
    order_desc = np.argsort(-counts, kind="stable")
    slot_of = {}   # expert -> (core, slot)
    expert_at = {}  # (core, slot) -> expert
    for r, e in enumerate(order_desc):
        c, j = (r, 0) if r < NCORES else (r - NCORES, 1)
        slot_of[int(e)] = (c, j)
        expert_at[(c, j)] = int(e)

    def _cap(es):
        return int(np.ceil(max(int(counts[es].max()), 128) / 128) * 128)

    Cs = [_cap(order_desc[:NCORES]), _cap(order_desc[NCORES:])]

    bf = ml_dtypes.bfloat16
    x_bf = x.astype(bf)
    W1_bf = W1.astype(bf)  # [E, D, H]
    W2_bf = W2.astype(bf)  # [E, H, D]

    idx_lists = [None] * E
    xts = [None] * E
    gs = [None] * E
    b1s = [None] * E
    for e in range(E):
        C = Cs[slot_of[e][1]]
        rows, which = np.nonzero(top_idx == e)
        idx_lists[e] = rows
        n_e = len(rows)
        xt = np.zeros((D, C), bf)
        xt[:, :n_e] = x_bf[rows].T
        xts[e] = xt
        gpad = np.zeros((C,), np.float32)
        gpad[:n_e] = top_gates[rows, which]
        gs[e] = np.ascontiguousarray(gpad.reshape(C // 128, 128).T)
        b1s[e] = np.ascontiguousarray(b1[e].reshape(H // 128, 128).T)

    # ---- build + compile per-core SPMD program ----
    nc, in_names, out_names = _build_program(Cs)

    in_maps = []
    for c in range(NCORES):
        m = {}
        for j in range(EPC):
            e = expert_at[(c, j)]
            m[in_names[f"w1_{j}"]] = W1_bf[e]
            m[in_names[f"w2_{j}"]] = W2_bf[e]
            m[in_names[f"xt_{j}"]] = xts[e]
            m[in_names[f"b1_{j}"]] = b1s[e]
            m[in_names[f"g_{j}"]] = gs[e]
        in_maps.append(m)

    res = run_bass_kernel_spmd(nc, in_maps, core_ids=list(range(NCORES)))
    LAST_RESULTS = res

    # ---- host combine ----
    gates_full = np.zeros((B, E), np.float32)
    gates_full[np.arange(B)[:, None], top_idx] = top_gates
    out = gates_full @ b2  # [B, D]
    for e in range(E):
        c, j = slot_of[e]
        y = np.asarray(res.results[c][out_names[f"y_{j}"]], np.float32)
        rows = idx_lists[e]
        out[rows] += y[: len(rows)]
    return out.astype(np.float32)


# revision 13
# speedup vs baseline: 1.1893x; 1.1893x over previous
"""MoE (noisy top-k gating, Shazeer-style) Trainium2 kernel.

Strategy (expert parallelism, per the sharding hint):
  - Gating (x@w_gate, noisy logits, top-4, softmax) runs on host in fp32
    numpy: it is 0.02% of the FLOPs and produces the routing needed to
    shard ("all-to-all dispatch" done host-side since I/O is full anyway).
  - The 16 experts' weights are sharded 2-per-core across 8 NeuronCores.
    Each core receives, per expert, the dispatched tokens x^T [D, C]
    (zero-padded to the global max expert load C), computes
    hT = relu(W1^T-free-dim-major matmul) and y = (hT)^T @ W2 scaled by
    the per-token gate, in bf16 on the tensor engine with fp32 PSUM
    accumulation.
  - Host scatters per-expert outputs back (each token appears in exactly
    its top-4 experts' outputs) and adds the gates @ b2 term.

Shapes are hardcoded for B=4096, D=1024, H=4096, E=16, TOP_K=4.
"""

import numpy as np
import ml_dtypes

import concourse.bass as bass
import concourse.mybir as mybir
import concourse.tile as tile
from concourse import bacc
from concourse.bass import ds, ts
from concourse.bass_utils import run_bass_kernel_spmd
from concourse.kernels.tile_matmul import (
    ShapeInfo,
    composable_matmul_tile_kernel,
    dma_from_dram_kxm,
    dma_from_dram_kxn,
    dma_to_dram_mxn,
    k_pool_min_bufs_for_dim,
)

B, D, H, E, TOP_K, NCORES = 4096, 1024, 4096, 16, 4, 8
EPC = E // NCORES  # experts per core
BF16 = mybir.dt.bfloat16
F32 = mybir.dt.float32
AF = mybir.ActivationFunctionType

# Results of the last device run (exec_time_ns etc.), for test harnesses.
LAST_RESULTS = None


def _gating(x, noise, w_gate, w_noise, b_noise):
    """Mirror of the reference gating in fp32 numpy.

    Verified on the actual inputs: the top-4 sets match jax-CPU bitwise
    selection (min 4th/5th logit gap 5.7e-5 vs <2e-6 numeric diff).
    """
    clean = x @ w_gate
    stddev = np.logaddexp(0.0, x @ w_noise + b_noise).astype(np.float32)
    noisy = clean + noise * stddev
    order = np.argsort(-noisy, axis=1, kind="stable")[:, :TOP_K]
    top_vals = np.take_along_axis(noisy, order, axis=1)
    ex = np.exp(top_vals - top_vals.max(axis=1, keepdims=True))
    top_gates = (ex / ex.sum(axis=1, keepdims=True)).astype(np.float32)
    return order, top_gates


def _relu_bias_reducer(b1_sb):
    def f(nc, psum, sbuf, md):
        hb = md.m_tile_idx * md.m_subtiles + md.m_subtile_idx
        nc.scalar.activation(
            sbuf[:, 0, :], psum[:], AF.Relu, bias=b1_sb[:, hb : hb + 1]
        )

    return f


def _gate_scale_reducer(g_sb):
    def f(nc, psum, sbuf, md):
        mb = md.m_tile_idx * md.m_subtiles + md.m_subtile_idx
        nc.scalar.activation(
            sbuf[:, 0, :], psum[:], AF.Copy, scale=g_sb[:, mb : mb + 1]
        )

    return f


def _noop_consumer(nc, t, md):
    pass


def _c_tile(C):
    for t in (512, 384, 256, 128):
        if C % t == 0:
            return t
    raise AssertionError(C)


def _build_program(Cs):
    """Build the SPMD per-core program: 2 experts, each a 2-layer FFN.

    Per expert: xT and the intermediate hT stay SBUF-resident; W1/W2
    stream from HBM; layer-1 output is written straight into the hT SBUF
    buffer (mxn_subtile_producer), layer 2 reads it as kxm via a custom
    producer — no DRAM roundtrip for h, no re-DMA of xT.
    """
    from contextlib import ExitStack

    nc = bacc.Bacc(None, target_bir_lowering=False)
    in_names = {}
    out_names = {}
    with ExitStack() as ctx:
        tc = ctx.enter_context(tile.TileContext(nc))
        dram = ctx.enter_context(tc.tile_pool(name="dram", bufs=1, space="DRAM"))
        const = ctx.enter_context(tc.tile_pool(name="const", bufs=1))

        ins = {}
        outs = {}
        for j in range(EPC):
            C = Cs[j]
            ins[f"w1_{j}"] = dram.tile([D, H], BF16, kind="ExternalInput", name=f"w1_{j}")
            ins[f"w2_{j}"] = dram.tile([H, D], BF16, kind="ExternalInput", name=f"w2_{j}")
            ins[f"xt_{j}"] = dram.tile([D, C], BF16, kind="ExternalInput", name=f"xt_{j}")
            ins[f"b1_{j}"] = dram.tile([128, H // 128], F32, kind="ExternalInput", name=f"b1_{j}")
            ins[f"g_{j}"] = dram.tile([128, C // 128], F32, kind="ExternalInput", name=f"g_{j}")
            outs[f"y_{j}"] = dram.tile([C, D], F32, kind="ExternalOutput", name=f"y_{j}")

        for key, ap in ins.items():
            in_names[key] = ap.tensor.name
        for key, ap in outs.items():
            out_names[key] = ap.tensor.name

        # Standing buffers for both experts' xT (and their W1 stream pools)
        # so the second expert's inputs prefetch with no SBUF-address WAR
        # against the first expert's in-flight reads.
        xt_sbs = []
        w1_pools = []
        for j in range(EPC):
            C = Cs[j]
            xt_sbs.append(const.tile([128, D // 128, C], BF16, name=f"xts{j}"))
            w1_pools.append(
                ctx.enter_context(
                    tc.tile_pool(name=f"w1p{j}", bufs=k_pool_min_bufs_for_dim(D) + 1)
                )
            )

        def _load_xt(j):
            C = Cs[j]
            CT = _c_tile(C)
            xt3d = ins[f"xt_{j}"].rearrange("(ko p) c -> p ko c", p=128)
            for cb in range(C // CT):
                cs = ds(cb * CT, CT)
                for kt in range(2):
                    ks = ds(kt * (D // 256), D // 256)
                    nc.sync.dma_start(xt_sbs[j][:, ks, cs], xt3d[:, ks, cs])

        _load_xt(0)

        for j in range(EPC):
            C = Cs[j]
            CT = _c_tile(C)  # exact N tile for layer 1
            b1_sb = const.tile([128, H // 128], F32, name=f"b1sb{j}")
            nc.sync.dma_start(b1_sb[:], ins[f"b1_{j}"][:])
            g_sb = const.tile([128, C // 128], F32, name=f"gsb{j}")
            nc.sync.dma_start(g_sb[:], ins[f"g_{j}"][:])

            with ExitStack() as ectx:
                pers = ectx.enter_context(
                    tc.tile_pool(name=f"pers{j}", bufs=1)
                )
                xt_sb = xt_sbs[j]
                hT_sb = pers.tile([128, H // 128, C], BF16, name=f"hts{j}")

                def xt_producer(nc_, md, xt_sb=xt_sb):
                    return xt_sb[
                        :,
                        ts(md.k_tile_idx, md.k_subtiles),
                        ds(md.n_tile_idx * md.n_tile, md.n_tile),
                    ]

                def hT_out_producer(nc_, md, hT_sb=hT_sb):
                    return hT_sb[
                        :,
                        ds(md.m_tile_idx * md.m_subtiles, md.m_subtiles),
                        ds(md.n_tile_idx * md.n_tile, md.n_tile),
                    ]

                def hT_kxm_producer(nc_, md, hT_sb=hT_sb):
                    return hT_sb[
                        :,
                        ts(md.k_tile_idx, md.k_subtiles),
                        ds(md.m_tile_idx * md.m_tile, md.m_tile),
                    ]

                # layer 1: hT[H, C] = relu(W1[D,H].T @ xT[D,C] + b1)
                tc.swap_default_side()
                with ExitStack() as mctx:
                    w1_producer, w1_shape = dma_from_dram_kxm(
                        w1_pools[j], ins[f"w1_{j}"][:]
                    )
                    composable_matmul_tile_kernel(
                        tc=tc,
                        kxm_shape=w1_shape,
                        kxn_shape=ShapeInfo(pdims=((128, D // 128),), fdims=(C,)),
                        output_type=BF16,
                        kxm_producer=w1_producer,
                        kxn_producer=xt_producer,
                        mxn_subtile_reducer=_relu_bias_reducer(b1_sb),
                        mxn_subtile_producer=hT_out_producer,
                        mxn_consumer=_noop_consumer,
                        MAX_TILE_SIZE=CT,
                        psum_n_bufs=2,
                    )

                # prefetch the next expert's xT while this expert computes
                if j + 1 < EPC:
                    _load_xt(j + 1)

                # layer 2: y[C, D] = g * (hT[H,C].T @ W2[H,D])
                tc.swap_default_side()
                with ExitStack() as mctx:
                    w2_pool = mctx.enter_context(
                        tc.tile_pool(
                            name=f"w2p{j}", bufs=k_pool_min_bufs_for_dim(H) + 1
                        )
                    )
                    w2_producer, w2_shape = dma_from_dram_kxn(
                        w2_pool, ins[f"w2_{j}"][:]
                    )
                    composable_matmul_tile_kernel(
                        tc=tc,
                        kxm_shape=ShapeInfo(pdims=((128, H // 128),), fdims=(C,)),
                        kxn_shape=w2_shape,
                        output_type=F32,
                        kxm_producer=hT_kxm_producer,
                        kxn_producer=w2_producer,
                        mxn_subtile_reducer=_gate_scale_reducer(g_sb),
                        mxn_consumer=dma_to_dram_mxn(outs[f"y_{j}"][:]),
                        psum_n_bufs=2,
                    )
    nc.compile()
    return nc, in_names, out_names


def kernel(x, noise, w_gate, w_noise, b_noise, W1, b1, W2, b2):
    global LAST_RESULTS
    x = np.asarray(x, np.float32)
    noise = np.asarray(noise, np.float32)
    w_gate = np.asarray(w_gate, np.float32)
    w_noise = np.asarray(w_noise, np.float32)
    b_noise = np.asarray(b_noise, np.float32)
    W1 = np.asarray(W1, np.float32)
    b1 = np.asarray(b1, np.float32)
    W2 = np.asarray(W2, np.float32)
    b2 = np.asarray(b2, np.float32)

    # ---- host gating + dispatch ----
    top_idx, top_gates = _gating(x, noise, w_gate, w_noise, b_noise)

    counts = np.bincount(top_idx.ravel(), minlength=E)

    # Slot assignment: rank experts by load; the 8 heaviest go to slot 0,
    # the 8 lightest to slot 1, so slot 1's padded capacity is smaller.
    order_desc = np.argsort(-counts, kind="stable")
    slot_of = {}   # expert -> (core, slot)
    expert_at = {}  # (core, slot) -> expert
    for r, e in enumerate(order_desc):
        c, j = (r, 0) if r < NCORES else (r - NCORES, 1)
        slot_of[int(e)] = (c, j)
        expert_at[(c, j)] = int(e)

    def _cap(es):
        return int(np.ceil(max(int(counts[es].max()), 128) / 128) * 128)

    Cs = [_cap(order_desc[:NCORES]), _cap(order_desc[NCORES:])]

    bf = ml_dtypes.bfloat16
    x_bf = x.astype(bf)
    W1_bf = W1.astype(bf)  # [E, D, H]
    W2_bf = W2.astype(bf)  # [E, H, D]

    idx_lists = [None] * E
    xts = [None] * E
    gs = [None] * E
    b1s = [None] * E
    for e in range(E):
        C = Cs[slot_of[e][1]]
        rows, which = np.nonzero(top_idx == e)
        idx_lists[e] = rows
        n_e = len(rows)
        xt = np.zeros((D, C), bf)
        xt[:, :n_e] = x_bf[rows].T
        xts[e] = xt
        gpad = np.zeros((C,), np.float32)
        gpad[:n_e] = top_gates[rows, which]
        gs[e] = np.ascontiguousarray(gpad.reshape(C // 128, 128).T)
        b1s[e] = np.ascontiguousarray(b1[e].reshape(H // 128, 128).T)

    # ---- build + compile per-core SPMD program ----
    nc, in_names, out_names = _build_program(Cs)

    in_maps = []
    for c in range(NCORES):
        m = {}
        for j in range(EPC):
            e = expert_at[(c, j)]
            m[in_names[f"w1_{j}"]] = W1_bf[e]
            m[in_names[f"w2_{j}"]] = W2_bf[e]
            m[in_names[f"xt_{j}"]] = xts[e]
            m[in_names[f"b1_{j}"]] = b1s[e]
            m[in_names[f"g_{j}"]] = gs[e]
        in_maps.append(m)

    res = run_bass_kernel_spmd(nc, in_maps, core_ids=list(range(NCORES)))
    LAST_RESULTS = res

    # ---- host combine ----
    gates_full = np.zeros((B, E), np.float32)
    gates_full[np.arange(B)[:, None], top_idx] = top_gates
    out = gates_full @ b2  # [B, D]
    for e in range(E):
        c, j = slot_of[e]
        y = np.asarray(res.results[c][out_names[f"y_{j}"]], np.float32)
        rows = idx_lists[e]
        out[rows] += y[: len(rows)]
    return out.astype(np.float32)
